# revision 1
# baseline (speedup 1.0000x reference)
"""Depth-aware average pooling (Wang & Neumann) on 8 Trainium2 NeuronCores.

  y[b,c,ho,wo] = (1/count) * sum_{kh,kw} exp(-|d[b,2ho+kh-1,2wo+kw-1] - d[b,2ho,2wo]|)
                                         * x[b,c,2ho+kh-1,2wo+kw-1]
  (3x3 window, stride 2, pad 1, padding positions excluded from count)

Sharding: data-parallel over batch (8 batches -> 8 cores).

Per-core design ("pair scheme"):
  - Output rows processed in pairs (ho=2p, 2p+1); SBUF partitions carry
    (g, c): g in {0,1} selects the pair half, c the 64 channels.
  - The 9 tap weights depend only on (ho, wo) and are shared across c.
    They are computed once in [ho-partition, 9*128] layout from depth,
    then broadcast across the (g,c) partitions on the TensorEngine.
    For exactness at bf16 matmul speed, the fp32 weights are split into
    3 bf16 terms (hi/mid/lo, residual encoding) and broadcast with a
    single K=6 selector matmul per pair (K-stacked terms) accumulating
    in fp32 PSUM -> exact to ~2^-26 relative.
  - DVE does the 9 per-tap multiplies + 8 accumulate adds per block.
  - Edge masking is free: depth tiles carry a 1e30 left-pad column /
    top-pad row, so exp(-|1e30 - d0|) underflows to exactly 0.
  - DMA issue cost (~1.5us per dma_start on a sequencer) is spread over
    the SP + Activation HWDGE queues and the GPSIMD SWDGE queue, and
    input tiles are loaded in 8-pair IO superblocks to halve DMA count.
"""

import os
import sys

sys.path.insert(0, "/opt/trn_rl_repo")

import numpy as np

B, C, H, W = 8, 64, 256, 256
Ho = Wo = 128
PB = 4  # pairs per compute block (PSUM-limited)
IOB = 8  # pairs per IO superblock
NBLK = (Ho // 2) // PB
BIG = 1e30

_CACHE = {}


def _build_nc(accum="dve", f32r=False, gps_adds=0, bench_reps=1):
    from contextlib import nullcontext

    import concourse.mybir as mybir
    import concourse.tile as tile
    from concourse import bacc

    f32 = mybir.dt.float32
    bf16 = mybir.dt.bfloat16
    AF = mybir.ActivationFunctionType

    nc = bacc.Bacc("TRN2", target_bir_lowering=False, debug=False)
    x = nc.dram_tensor("x", [C, H, W], f32, kind="ExternalInput")
    d = nc.dram_tensor("d", [H, W], f32, kind="ExternalInput")
    y = nc.dram_tensor("y", [C, Ho, Wo], f32, kind="ExternalOutput")

    with tile.TileContext(nc) as tc:
        with (
            tc.tile_pool(name="const", bufs=1) as cpool,
            tc.tile_pool(name="wtmp", bufs=2) as tpool,
            tc.tile_pool(name="xin", bufs=2) as xpool,
            tc.tile_pool(name="w2", bufs=2) as w2pool,
            tc.tile_pool(name="prod", bufs=4) as ppool,
            tc.tile_pool(name="outsb", bufs=2) as opool,
            tc.tile_pool(name="wpsum", bufs=2, space="PSUM") as wpool,
        ):
            # ---------------- depth tiles + tap weights ----------------
            Dm1 = cpool.tile([128, 257], f32)  # row 2ho-1
            D0 = cpool.tile([128, 257], f32)  # row 2ho
            Dp1 = cpool.tile([128, 257], f32)  # row 2ho+1
            dre = d.rearrange("(h t) w -> h t w", t=2)  # [128,2,256]

            nc.vector.memset(Dm1[:, 0:1], BIG)
            nc.vector.memset(Dm1[0:1, :], BIG)  # row -1 (ho=0)
            nc.gpsimd.dma_start(out=Dm1[1:128, 1:257], in_=dre[0:127, 1, :])
            nc.vector.memset(D0[:, 0:1], BIG)
            nc.gpsimd.dma_start(out=D0[:, 1:257], in_=dre[:, 0, :])
            nc.vector.memset(Dp1[:, 0:1], BIG)
            nc.gpsimd.dma_start(out=Dp1[:, 1:257], in_=dre[:, 1, :])

            # W9[ho, k*128 + wo] = exp(-|dp_k - d0|) / count
            W9 = cpool.tile([128, 1152], f32)
            rc = cpool.tile([128, 128], f32)
            nc.vector.memset(rc[:, :], 1.0 / 9.0)
            nc.vector.memset(rc[:, 0:1], 1.0 / 6.0)
            nc.vector.memset(rc[0:1, :], 1.0 / 6.0)
            nc.vector.memset(rc[0:1, 0:1], 1.0 / 4.0)

            d0 = D0[:, 1:257:2]
            for kh, Dk in enumerate((Dm1, D0, Dp1)):
                for kw in range(3):
                    k = kh * 3 + kw
                    t = tpool.tile([128, 128], f32, name=f"t{k}", tag="wt")
                    nc.vector.tensor_sub(t, Dk[:, kw : kw + 255 : 2], d0)
                    nc.scalar.activation(t, t, AF.Abs)
                    nc.scalar.activation(
                        W9[:, k * 128 : (k + 1) * 128], t, AF.Exp, scale=-1.0
                    )
            for k in range(9):
                sl = W9[:, k * 128 : (k + 1) * 128]
                nc.vector.tensor_mul(sl, sl, rc)

            # exact 3-term bf16 residual split of W9: W9 = hi + mid + lo
            Whi = cpool.tile([128, 1152], bf16)
            Wmid = cpool.tile([128, 1152], bf16)
            Wlo = cpool.tile([128, 1152], bf16)
            r1 = tpool.tile([128, 1152], f32, name="r1", tag="res")
            r2 = tpool.tile([128, 1152], f32, name="r2", tag="res")
            hi32 = tpool.tile([128, 1152], f32, name="hi32", tag="res32")
            nc.vector.tensor_copy(Whi, W9)  # f32 -> bf16 (RNE)
            nc.scalar.copy(hi32, Whi)  # back to f32
            nc.vector.tensor_sub(r1, W9, hi32)
            nc.vector.tensor_copy(Wmid, r1)
            nc.scalar.copy(hi32, Wmid)
            nc.vector.tensor_sub(r2, r1, hi32)
            nc.vector.tensor_copy(Wlo, r2)

            # selector [6, 128] bf16: rows (2t+g): 1 on partitions of half g
            sel3 = cpool.tile([6, 128], bf16)
            nc.vector.memset(sel3[:, :], 0.0)
            nc.vector.memset(sel3[0:1, 0:64], 1.0)
            nc.gpsimd.dma_start(out=sel3[1:2, 64:128], in_=sel3[0:1, 0:64])
            nc.gpsimd.dma_start(out=sel3[2:4, :], in_=sel3[0:2, :])
            nc.gpsimd.dma_start(out=sel3[4:6, :], in_=sel3[0:2, :])

            # ---------------- main loop ----------------
            xre = x.rearrange("c (p r) w -> c p r w", r=4)  # [64,64,4,256]
            yre = y.rearrange("c (q g) w -> g c q w", g=2)  # [2,64,64,128]

            rep_cm = tc.For_i(0, bench_reps, 1) if bench_reps > 1 else nullcontext()
            with rep_cm:
                _main_body(nc, xpool, w2pool, ppool, opool, wpool,
                           Whi, Wmid, Wlo, sel3, xre, yre, f32, bf16, gps_adds)
    nc.compile()
    return nc


def _main_body(nc, xpool, w2pool, ppool, opool, wpool,
               Whi, Wmid, Wlo, sel3, xre, yre, f32, bf16, gps_adds=0):
    if True:
        if True:
            for iob in range(Ho // 2 // IOB):
                P0 = iob * IOB

                # weight rows restaged to parity layout (base partition 0)
                W2 = w2pool.tile([6, IOB, 1152], bf16, name="W2", tag="W2")
                for t_i, (Wt, eng) in enumerate(
                    zip((Whi, Wmid, Wlo), (nc.sync, nc.scalar, nc.gpsimd))
                ):
                    for g in range(2):
                        eng.dma_start(
                            out=W2[2 * t_i + g : 2 * t_i + g + 1, :, :],
                            in_=Wt[2 * P0 + g : 2 * P0 + 2 * IOB + g - 1 : 2, :],
                        )

                # input tiles: T[kh][(g,c), pair, 1+256] holding rows
                # 2(2p+g)-1+kh for the IOB pairs of this superblock
                T = [
                    xpool.tile([128, IOB, 257], f32, name=f"T{i}", tag=f"T{i}")
                    for i in range(3)
                ]
                nc.gpsimd.memset(T[0][:, :, 0:1], 0.0)
                nc.gpsimd.memset(T[1][:, :, 0:1], 0.0)
                nc.gpsimd.memset(T[2][64:128, :, 0:1], 0.0)
                # T1: rows 4p (h0), 4p+2 (h1)
                nc.sync.dma_start(out=T[1][0:64, :, 1:257], in_=xre[:, P0 : P0 + IOB, 0, :])
                nc.sync.dma_start(out=T[1][64:128, :, 1:257], in_=xre[:, P0 : P0 + IOB, 2, :])
                # T0: rows 4p-1 (h0), 4p+1 (h1)
                if iob == 0:
                    nc.sync.dma_start(out=T[0][0:64, 0:1, 1:257], in_=xre[:, 0:1, 1, :])
                    nc.sync.dma_start(
                        out=T[0][0:64, 1:IOB, 1:257], in_=xre[:, 0 : IOB - 1, 3, :]
                    )
                else:
                    nc.sync.dma_start(
                        out=T[0][0:64, :, 1:257], in_=xre[:, P0 - 1 : P0 + IOB - 1, 3, :]
                    )
                nc.sync.dma_start(out=T[0][64:128, :, 1:257], in_=xre[:, P0 : P0 + IOB, 1, :])
                # T2: rows 4p+1 (h0, SBUF dup of T0 h1), 4p+3 (h1)
                nc.sync.dma_start(out=T[2][0:64, :, :], in_=T[0][64:128, :, :])
                nc.scalar.dma_start(out=T[2][64:128, :, 1:257], in_=xre[:, P0 : P0 + IOB, 3, :])

                osb = opool.tile([128, IOB, 128], f32, name="osb", tag="osb")
                if gps_adds:
                    osbB = opool.tile([128, IOB, 128], f32, name="osbB", tag="osbB")

                for cb in range(IOB // PB):  # compute blocks within superblock
                    q0 = cb * PB
                    acc = osb[:, q0 : q0 + PB, :]
                    # chain B: the last (gps_adds+1) products are summed on
                    # GPSIMD (contiguous operands only); chain A stays on DVE
                    nB = gps_adds + 1 if gps_adds else 0
                    accB = osbB[:, q0 : q0 + PB, :] if gps_adds else None
                    nb_seen = 0
                    na_seen = 0
                    # chain-B products (GPSIMD-summed) are produced first so
                    # the GPSIMD adds overlap DVE's chain-A work
                    g_order = (2, 0, 1) if gps_adds else (0, 1, 2)
                    for g in g_order:  # tap group == kh
                        # one PSUM bank per pair slice (512 f32)
                        PW = wpool.tile(
                            [128, PB, 512], f32, name=f"PW{g}", tag="PW", space="PSUM"
                        )
                        for q in range(PB):
                            nc.tensor.matmul(
                                PW[:, q, 0:384],
                                sel3[:, :],
                                W2[:, q0 + q, g * 384 : (g + 1) * 384],
                                start=True,
                                stop=True,
                            )
                        for kw in range(3):
                            k = g * 3 + kw
                            x_ap = T[g][:, q0 : q0 + PB, kw : kw + 255 : 2]
                            w_ap = PW[:, :, kw * 128 : (kw + 1) * 128]
                            if nb_seen < nB:
                                # product for chain B (DVE mult, GPSIMD add)
                                P = ppool.tile(
                                    [128, PB, 128], f32, name=f"Pb{k}", tag="Pb",
                                    bufs=4,
                                )
                                nc.vector.tensor_mul(P, x_ap, w_ap)
                                # chain with fresh destinations (no in-place
                                # GPSIMD accumulate); last add lands in accB
                                if nb_seen == 0:
                                    chainB_prev = P
                                else:
                                    if nb_seen == nB - 1:
                                        dst = accB
                                    else:
                                        dst = ppool.tile(
                                            [128, PB, 128], f32,
                                            name=f"Bacc{k}", tag="Bacc", bufs=4,
                                        )
                                    nc.gpsimd.tensor_add(dst, chainB_prev, P)
                                    chainB_prev = dst
                                nb_seen += 1
                            elif na_seen == 0:
                                nc.vector.tensor_mul(acc, x_ap, w_ap)
                                na_seen += 1
                            else:
                                P = ppool.tile(
                                    [128, PB, 128], f32, name=f"P{k}", tag="P"
                                )
                                nc.vector.tensor_mul(P, x_ap, w_ap)
                                nc.vector.tensor_add(acc, acc, P)
                                na_seen += 1

                if gps_adds:
                    # single decoupled merge of the GPSIMD branch per superblock
                    nc.vector.tensor_add(
                        osb.rearrange("p q w -> p (q w)"),
                        osb.rearrange("p q w -> p (q w)"),
                        osbB.rearrange("p q w -> p (q w)"),
                    )
                for g in range(2):
                    nc.scalar.dma_start(
                        out=yre[g, :, P0 : P0 + IOB, :],
                        in_=osb[64 * g : 64 * (g + 1), :, :],
                    )


def _get_nc():
    accum = os.environ.get("DAP_ACCUM", "dve")
    f32r = os.environ.get("DAP_F32R", "0") == "1"
    gps = int(os.environ.get("DAP_GPS_ADDS", "0"))
    key = (accum, f32r, gps)
    if key not in _CACHE:
        _CACHE[key] = _build_nc(accum=accum, f32r=f32r, gps_adds=gps)
    return _CACHE[key]


def run_spmd(input, depth, trace=False):
    from concourse.bass_utils import run_bass_kernel_spmd

    nc = _get_nc()
    input = np.ascontiguousarray(input, dtype=np.float32)
    depth = np.ascontiguousarray(depth, dtype=np.float32)
    in_maps = [{"x": input[b], "d": depth[b, 0]} for b in range(B)]
    res = run_bass_kernel_spmd(nc, in_maps, core_ids=list(range(B)), trace=trace)
    out = np.stack([res.results[b]["y"] for b in range(B)])
    return out, res


def kernel(input, depth):
    out, _ = run_spmd(input, depth, trace=False)
    return out

